# revision 1
# baseline (speedup 1.0000x reference)
"""Sparse Adagrad (Habana-style) on 8 Trainium2 NeuronCores.

Strategy: row-shard the embedding tables (weights/moments) across the 8
cores by index range (62500 rows each, padded to 63488 = 128*496). The
host routes each valid gradient row to its owning core. On device, each
core sweeps its table shard once with large contiguous DMAs; the sparse
scatter-add (with duplicate indices) is done with one-hot matmuls on the
TensorEngine accumulating into PSUM, so duplicates sum natively.

Table layout per core: row r -> SBUF partition p = r // 496, free offset
j = r % 496 (so a [63488, 64] f32 shard is exactly a [128, 496*64] SBUF
sweep with contiguous per-partition DMA).

Per block j (the 128 rows {p*496 + j}), the host packs the gradient rows
whose local index maps to block j into up to CPB chunks of 128 "slots"
(slot -> partition). A one-hot matrix A[slot, p] = (strip(slot) == p)
is built on device via is_equal against an iota, and
    psum_m[p, :] += A.T @ g2_chunk      (moment increments, Sum g^2)
    psum_g[p, :] += A.T @ g_chunk       (gradient sums, Sum g)
Then the update (denominator uses the fully accumulated moment, and it
is constant across duplicates so it factors out of the sum):
    m' = m + psum_m
    w' = w - lr * psum_g / sqrt(m' + 1e-20)
"""

import sys

for _p in ("/opt/trn_rl_repo", "/root/.axon_site/_ro/trn_rl_repo"):
    if _p not in sys.path:
        sys.path.insert(0, _p)

import numpy as np

P = 128          # SBUF partitions / matmul contraction
D = 64           # embedding dim
NCORES = 8
VC = 62500       # table rows per core
R = 496          # rows per strip (= blocks per core); 128*496 = 63488 >= VC
PADV = P * R     # padded rows per core
JSUB = 16        # blocks per sweep iteration (PSUM limited)
NIT = R // JSUB  # 31 sweep iterations

_program_cache = {}


def _build_program(cpb, cap, jsub=JSUB, sbufs=3, pbufs=2, store_engine="scalar",
                   g_dtype="fp16", g_load_engine="sync", reps=1,
                   loop_reps=False):
    from concourse import bacc, mybir
    import concourse.tile as tile

    nit = R // jsub
    assert nit * jsub == R
    f32 = mybir.dt.float32
    nc = bacc.Bacc("TRN2", target_bir_lowering=False, debug=False,
                   num_devices=NCORES)

    w_in = nc.dram_tensor("w_in", [P, R * D], f32, kind="ExternalInput")
    m_in = nc.dram_tensor("m_in", [P, R * D], f32, kind="ExternalInput")
    gdt = {"f32": f32, "bf16": mybir.dt.bfloat16,
           "fp16": mybir.dt.float16}[g_dtype]
    g_in = nc.dram_tensor("g_in", [cap, R * cpb * D], gdt,
                          kind="ExternalInput")
    midx = nc.dram_tensor("midx", [cap, R * cpb], f32, kind="ExternalInput")
    lr_in = nc.dram_tensor("lr", [1, 1], f32, kind="ExternalInput")
    w_out = nc.dram_tensor("w_out", [P, R * D], f32, kind="ExternalOutput")
    m_out = nc.dram_tensor("m_out", [P, R * D], f32, kind="ExternalOutput")

    with tile.TileContext(nc) as tc:
        with tc.tile_pool(name="consts", bufs=1) as consts, \
             tc.tile_pool(name="sbuf", bufs=sbufs) as pool, \
             tc.tile_pool(name="psum", bufs=pbufs, space="PSUM") as psum:
            iota_i = consts.tile([P, P], mybir.dt.int32)
            nc.gpsimd.iota(iota_i[:], pattern=[[1, P]], base=0,
                           channel_multiplier=0)
            iota_f = consts.tile([P, P], f32)
            nc.vector.tensor_copy(iota_f[:], iota_i[:])

            eps_t = consts.tile([P, 1], f32)
            nc.gpsimd.memset(eps_t[:], 1e-20)

            # inv_s2 = 1 / stream_scale^2 (precomputed on host), used to
            # recover Sum g^2 from Sum (stream_scale*g)^2
            inv_s2 = consts.tile([P, 1], f32)
            nc.sync.dma_start(out=inv_s2[:], in_=lr_in[:].to_broadcast((P, 1)))

            midx_s = consts.tile([cap, R * cpb], f32)
            nc.sync.dma_start(out=midx_s[:], in_=midx[:])

            store = getattr(nc, store_engine)

            import contextlib

            def _rep_scope():
                if loop_reps and reps > 1:
                    return tc.For_i(0, reps, 1)
                return contextlib.nullcontext()

            with _rep_scope():
              for _rep in range(1 if loop_reps else reps):
                for it in range(nit):
                    c0, c1 = it * jsub * D, (it + 1) * jsub * D
                    s0, s1 = it * jsub * cpb * D, (it + 1) * jsub * cpb * D
                    k0 = it * jsub * cpb

                    w_s = pool.tile([P, jsub * D], f32)
                    nc.sync.dma_start(out=w_s[:], in_=w_in[:, c0:c1])
                    m_s = pool.tile([P, jsub * D], f32)
                    nc.sync.dma_start(out=m_s[:], in_=m_in[:, c0:c1])
                    g_s = pool.tile([cap, jsub * cpb * D], gdt)
                    getattr(nc, g_load_engine).dma_start(out=g_s[:],
                                                         in_=g_in[:, s0:s1])

                    g2_s = pool.tile([cap, jsub * cpb * D], gdt)
                    nc.scalar.square(g2_s[:], g_s[:])

                    a_s = pool.tile([cap, jsub * cpb, P], gdt)
                    nc.vector.tensor_tensor(
                        out=a_s[:],
                        in0=midx_s[:, k0:k0 + jsub * cpb, None].broadcast_to(
                            (cap, jsub * cpb, P)),
                        in1=iota_f[:cap, None, :].broadcast_to(
                            (cap, jsub * cpb, P)),
                        op=mybir.AluOpType.is_equal,
                    )

                    psum_m = psum.tile([P, jsub * D], f32)
                    psum_g = psum.tile([P, jsub * D], f32)
                    for jj in range(jsub):
                        for c in range(cpb):
                            k = jj * cpb + c
                            nc.tensor.matmul(
                                out=psum_m[:, jj * D:(jj + 1) * D],
                                lhsT=a_s[:, k, :],
                                rhs=g2_s[:, k * D:(k + 1) * D],
                                start=(c == 0), stop=(c == cpb - 1),
                            )
                        for c in range(cpb):
                            k = jj * cpb + c
                            nc.tensor.matmul(
                                out=psum_g[:, jj * D:(jj + 1) * D],
                                lhsT=a_s[:, k, :],
                                rhs=g_s[:, k * D:(k + 1) * D],
                                start=(c == 0), stop=(c == cpb - 1),
                            )

                    m_n = pool.tile([P, jsub * D], f32)
                    nc.vector.affine_then_add(out=m_n[:], in0=psum_m[:],
                                              in1=m_s[:], scale=inv_s2[:],
                                              bias=0.0)
                    store.dma_start(out=m_out[:, c0:c1], in_=m_n[:])

                    s_t = pool.tile([P, jsub * D], f32)
                    nc.scalar.activation(s_t[:], m_n[:],
                                         mybir.ActivationFunctionType.Sqrt,
                                         bias=eps_t[:])
                    r_t = pool.tile([P, jsub * D], f32)
                    nc.vector.reciprocal_approx_fast(out=r_t[:], in_=s_t[:])
                    t_t = pool.tile([P, jsub * D], f32)
                    nc.vector.tensor_mul(t_t[:], r_t[:], psum_g[:])
                    w_n = pool.tile([P, jsub * D], f32)
                    nc.gpsimd.tensor_tensor(out=w_n[:], in0=w_s[:], in1=t_t[:],
                                            op=mybir.AluOpType.add)
                    store.dma_start(out=w_out[:, c0:c1], in_=w_n[:])

    nc.compile()
    return nc


def get_program(cpb, cap, **opts):
    key = (cpb, cap, tuple(sorted(opts.items())))
    if key not in _program_cache:
        _program_cache[key] = _build_program(cpb, cap, **opts)
    return _program_cache[key]


def prepare_inputs(gradients, weights, moments, indices, learning_rate,
                   valid_count, g_dtype="fp16"):
    """Host-side routing: shard tables by row range, route gradient rows to
    owning cores, pack into the block/slot layout the device sweep expects."""
    g = np.ascontiguousarray(np.asarray(gradients, dtype=np.float32))
    w = np.asarray(weights, dtype=np.float32)
    m = np.asarray(moments, dtype=np.float32)
    idx = np.asarray(indices).astype(np.int64)
    vc = int(valid_count)
    lr = np.float32(np.asarray(learning_rate).reshape(-1)[0])

    idxv = idx[:vc]
    owner = idxv // VC
    loc = idxv - owner * VC
    j = loc % R
    mstrip = loc // R

    group = owner * R + j
    counts = np.bincount(group, minlength=NCORES * R)
    order = np.argsort(group, kind="stable")
    starts = np.concatenate(([0], np.cumsum(counts)[:-1]))
    rank = np.empty(vc, dtype=np.int64)
    rank[order] = np.arange(vc, dtype=np.int64) - starts[group[order]]

    maxcnt = max(1, int(counts.max()))
    cap = min(P, -(-maxcnt // 16) * 16)  # chunk capacity, multiple of 16
    cpb = -(-maxcnt // cap)              # chunks per block

    colidx = j * cpb + rank // cap
    part = rank % cap

    if g_dtype == "bf16":
        import ml_dtypes
        np_gdt = ml_dtypes.bfloat16
    elif g_dtype == "fp16":
        np_gdt = np.float16
    else:
        np_gdt = np.float32
    sscale = -lr if lr != 0.0 else 1.0
    g_dev = np.zeros((NCORES, cap, R * cpb, D), dtype=np_gdt)
    g_dev[owner, part, colidx] = (np.float32(sscale) * g[:vc]).astype(np_gdt)
    g_dev = g_dev.reshape(NCORES, cap, R * cpb * D)

    midx_dev = np.zeros((NCORES, cap, R * cpb), dtype=np.float32)
    midx_dev[owner, part, colidx] = mstrip.astype(np.float32)

    w_dev = np.zeros((NCORES, PADV, D), dtype=np.float32)
    w_dev[:, :VC] = w.reshape(NCORES, VC, D)
    w_dev = w_dev.reshape(NCORES, P, R * D)
    m_dev = np.zeros((NCORES, PADV, D), dtype=np.float32)
    m_dev[:, :VC] = m.reshape(NCORES, VC, D)
    m_dev = m_dev.reshape(NCORES, P, R * D)

    lr_arr = np.full((1, 1), 1.0 / (sscale * sscale), dtype=np.float32)

    in_maps = [
        {
            "w_in": w_dev[c],
            "m_in": m_dev[c],
            "g_in": g_dev[c],
            "midx": midx_dev[c],
            "lr": lr_arr,
        }
        for c in range(NCORES)
    ]
    return in_maps, cpb, cap


def assemble_outputs(results):
    w_new = np.empty((NCORES * VC, D), dtype=np.float32)
    m_new = np.empty((NCORES * VC, D), dtype=np.float32)
    for c in range(NCORES):
        w_new[c * VC:(c + 1) * VC] = \
            results[c]["w_out"].reshape(PADV, D)[:VC]
        m_new[c * VC:(c + 1) * VC] = \
            results[c]["m_out"].reshape(PADV, D)[:VC]
    return w_new, m_new


def kernel(gradients, weights, moments, indices, learning_rate, valid_count):
    from concourse.bass_utils import run_bass_kernel_spmd

    lr = float(np.asarray(learning_rate).reshape(-1)[0])
    if lr == 0.0:
        # Degenerate case (never hit with this spec): weights unchanged,
        # moments still accumulate g^2. Compute on host.
        g = np.asarray(gradients, dtype=np.float32).copy()
        g[int(valid_count):] = 0.0
        idx = np.asarray(indices).astype(np.int64)
        m_new = np.asarray(moments, dtype=np.float32).copy()
        np.add.at(m_new, idx, g * g)
        return np.asarray(weights, dtype=np.float32).copy(), m_new

    in_maps, cpb, cap = prepare_inputs(gradients, weights, moments, indices,
                                       learning_rate, valid_count)
    nc = get_program(cpb, cap)
    res = run_bass_kernel_spmd(nc, in_maps, core_ids=list(range(NCORES)))
    return assemble_outputs(res.results)



# revision 2
# speedup vs baseline: 1.0566x; 1.0566x over previous
"""Sparse Adagrad (Habana-style) on 8 Trainium2 NeuronCores.

Row-shard the tables across 8 cores by index range (62500 rows each).
Only the TOUCHED rows (~20.6k per core, ~33%) are shipped to the
device, compacted into a [128 partitions x RP blocks] layout chosen by
the host; untouched rows pass through on the host. All device traffic
is fp16 (tolerance is 2e-2; we land ~1e-3).

Compact layout: touched rows are sorted by duplicate-count (desc) and
snake-dealt across RP blocks; the i-th dealt row lands at block
j = snake(i % RP), height h = i // RP, i.e. table position
(partition h, column j). Each row's FIRST gradient occurrence is
stored at base slot h of block j, so the base scatter matrix is the
IDENTITY (constant). Duplicate occurrences (~4.4k/core) are pooled per
4-block PSUM-bank group (up to 128 slots) with a one-hot A_ovf built
on device via is_equal against an iota.

Per PSUM bank (4 blocks, psum[:, 4b:4b+4, 0:128] with [Sum g | Sum g2]
halves per block):
    bank = I @ gsq[4 blocks]            (identity matmul, start=True)
    bank += A_ovf[grp] @ go_diag[grp]   (block-diagonal overflow rhs,
                                         stop=True, same footprint —
                                         HW requires accumulation
                                         groups to open/close with
                                         identical out regions)
then
    m'  = m + Sum g2                          (DVE; GPSIMD can't read PSUM)
    r   = AbsRsqrt(m'*(1/lr^2) + eps)         (ACT)  [= lr*rsqrt(m')]
    u   = r * Sum g                           (DVE)  [= lr*Sum g/sqrt(m')]
outputs [u | m'] per row; the host applies w' = w - u in f32 during
assembly (w never round-trips through fp16). The denominator uses the
fully accumulated m' and is constant across duplicates, so it factors
out of the sum — matching the reference exactly.
"""

import sys

for _p in ("/opt/trn_rl_repo", "/root/.axon_site/_ro/trn_rl_repo"):
    if _p not in sys.path:
        sys.path.insert(0, _p)

import numpy as np

P = 128          # SBUF partitions
D = 64           # embedding dim
NCORES = 8
VC = 62500       # table rows per core
OVF = 32         # overflow slots per block
JSUB = 12        # blocks per sweep iteration (PSUM: [128, 12*128] f32 = 3 banks)

_program_cache = {}


def _build_program(rp, reps=1, rsqrt='act'):
    """rp: number of blocks (table columns) per core; rp % 12 == 0, % 4 == 0."""
    from concourse import bacc, mybir
    import concourse.tile as tile

    nit = rp // JSUB
    assert nit * JSUB == rp and rp % 4 == 0
    rp4 = rp // 4
    f32 = mybir.dt.float32
    f16 = mybir.dt.float16
    nc = bacc.Bacc("TRN2", target_bir_lowering=False, debug=False,
                   num_devices=NCORES)

    # moments only on device; host applies w' = w - u during assembly
    m_in = nc.dram_tensor("m_in", [P, rp * D], f16, kind="ExternalInput")
    g_in = nc.dram_tensor("g_in", [P, rp * D], f16, kind="ExternalInput")
    go_in = nc.dram_tensor("go_in", [P, rp4 * 4 * 2 * D], f16,
                          kind="ExternalInput")
    midxo = nc.dram_tensor("midxo", [P, rp4], f16, kind="ExternalInput")
    scal = nc.dram_tensor("scal", [1, 2], f32, kind="ExternalInput")  # [inv_lr2, eps]
    # output: [u | m'] interleaved per block
    um_out = nc.dram_tensor("um_out", [P, rp * 2 * D], f16,
                            kind="ExternalOutput")

    with tile.TileContext(nc) as tc:
        with tc.tile_pool(name="consts", bufs=1) as consts, \
             tc.tile_pool(name="big", bufs=2) as bigpool, \
             tc.tile_pool(name="sbuf", bufs=4) as pool, \
             tc.tile_pool(name="psum", bufs=2, space="PSUM") as psum:
            # iota along free dim (same in every partition), fp16
            iota_i = consts.tile([P, P], mybir.dt.int32)
            nc.gpsimd.iota(iota_i[:], pattern=[[1, P]], base=0,
                           channel_multiplier=0)
            iota_f = consts.tile([P, P], f16)
            nc.vector.tensor_copy(iota_f[:], iota_i[:])
            # partition index (one value per partition), fp16
            piota_i = consts.tile([P, 1], mybir.dt.int32)
            nc.gpsimd.iota(piota_i[:], pattern=[[1, 1]], base=0,
                           channel_multiplier=1)
            piota_f = consts.tile([P, 1], f16)
            nc.vector.tensor_copy(piota_f[:], piota_i[:])
            # identity matrix [p, f] = (f == p), fp16
            ident = consts.tile([P, P], f16)
            nc.vector.tensor_tensor(
                out=ident[:],
                in0=iota_f[:],
                in1=piota_f[:].to_broadcast((P, P)),
                op=mybir.AluOpType.is_equal,
            )

            inv_lr2 = consts.tile([P, 1], f32)
            nc.sync.dma_start(out=inv_lr2[:],
                              in_=scal[:, 0:1].to_broadcast((P, 1)))
            eps_t = consts.tile([P, 1], f32)
            nc.sync.dma_start(out=eps_t[:],
                              in_=scal[:, 1:2].to_broadcast((P, 1)))

            # overflow gradients, block-diagonal per 4-block group:
            # go_s[slot, grp, db, 0:64]=g, [64:128]=g^2 of that slot if it
            # belongs to sub-block db, else zero. Resident all sweep.
            go_s = consts.tile([P, rp4, 4, 2 * D], f16)
            nc.sync.dma_start(out=go_s[:], in_=go_in[:])
            midxo_s = consts.tile([P, rp4], f16)
            nc.sync.dma_start(out=midxo_s[:], in_=midxo[:])

            # A_ovf[slot, grp, p] = (midxo[slot, grp] == p)
            a_ovf = consts.tile([P, rp4, P], f16)
            nc.vector.tensor_tensor(
                out=a_ovf[:],
                in0=midxo_s[:, :, None].broadcast_to((P, rp4, P)),
                in1=iota_f[:, None, :].broadcast_to((P, rp4, P)),
                op=mybir.AluOpType.is_equal,
            )

            import contextlib

            def _rep_scope():
                return contextlib.nullcontext()

            with _rep_scope():
              for _rep in range(reps):
                NH = 2
                for it2 in range(nit // NH):
                    # DMA at 2-iteration granularity — bigger transfers
                    # amortize DGE latency while staying fine-grained enough
                    # to overlap with compute; compute stays at JSUB blocks
                    # per step (PSUM size).
                    J2 = NH * JSUB
                    j00 = it2 * J2
                    m2 = bigpool.tile([P, NH, JSUB, D], f16)
                    nc.sync.dma_start(
                        out=m2[:], in_=m_in[:, j00 * D:(j00 + J2) * D])
                    gb2 = bigpool.tile([P, NH, JSUB * D], f16)
                    nc.sync.dma_start(
                        out=gb2[:], in_=g_in[:, j00 * D:(j00 + J2) * D])
                    um2_n = bigpool.tile([P, NH, JSUB, 2 * D], f16)
                    for half in range(NH):
                        j0 = j00 + half * JSUB

                        # [g | g^2] rhs tile: ACT fills both halves
                        gsq = pool.tile([P, JSUB, 2 * D], f16)
                        gb_v = gb2[:, half].rearrange("p (j d) -> p j d",
                                                      j=JSUB)
                        nc.scalar.copy(gsq[:, :, 0:D], gb_v)
                        nc.gpsimd.tensor_tensor(
                            out=gsq[:, :, D:2 * D], in0=gb_v, in1=gb_v,
                            op=mybir.AluOpType.mult)

                        ps = psum.tile([P, JSUB, 2 * D], f32)
                        # Per-region accumulation groups (open and close with
                        # the SAME out footprint — HW/NEFF rejects mismatched
                        # group shapes). start=True lazily marks the whole
                        # 2KB bank pending-zero, so each region's overflow
                        # accumulate must land before the next start touches
                        # that bank: waves of 3 regions in 3 distinct banks
                        # {w, w+4, w+8}, which also share the identity
                        # stationary across 3 matmuls (alternating weights
                        # cost ~3x on PE).
                        # HW requires accumulation groups to open and
                        # close with the SAME out footprint: both the base
                        # (identity) and overflow matmuls cover one whole
                        # PSUM bank (4 blocks, N=512). The overflow rhs is
                        # block-diagonal so one 128-slot matmul serves the
                        # bank's 4 blocks.
                        for b in range(3):
                            nc.tensor.matmul(
                                out=ps[:, 4 * b:4 * (b + 1), :],
                                lhsT=ident[:],
                                rhs=gsq[:, 4 * b:4 * (b + 1), :],
                                start=True, stop=False,
                                skip_group_check=True,
                            )
                        for b in range(3):
                            grp = j0 // 4 + b
                            nc.tensor.matmul(
                                out=ps[:, 4 * b:4 * (b + 1), :],
                                lhsT=a_ovf[:, grp, :],
                                rhs=go_s[:, grp, :, :],
                                start=False, stop=True,
                                skip_group_check=True,
                            )

                        # m' = m + Sum g^2  (psum high half; GPSIMD cannot
                        # read PSUM, so this lives on DVE)
                        nc.vector.tensor_tensor(
                            out=um2_n[:, half, :, D:2 * D],
                            in0=ps[:, :, D:2 * D],
                            in1=m2[:, half],
                            op=mybir.AluOpType.add,
                        )
                        # r = 1/sqrt(m'*inv_lr2 + eps) [= lr*rsqrt(m')]
                        # in one ACT op; input >= 0 so the |x| is a no-op.
                        # (CoreSim lacks the fused op: rsqrt='split' swaps in
                        # the equivalent Sqrt + reciprocal chain for sim.)
                        r_t = pool.tile([P, JSUB, D], f32)
                        if rsqrt == 'act':
                            nc.scalar.activation(
                                r_t[:], um2_n[:, half, :, D:2 * D],
                                mybir.ActivationFunctionType.
                                Abs_reciprocal_sqrt,
                                bias=eps_t[:], scale=inv_lr2[:])
                        else:
                            s_t = pool.tile([P, JSUB, D], f32)
                            nc.scalar.activation(
                                s_t[:], um2_n[:, half, :, D:2 * D],
                                mybir.ActivationFunctionType.Sqrt,
                                bias=eps_t[:], scale=inv_lr2[:])
                            nc.vector.reciprocal_approx_fast(out=r_t[:],
                                                             in_=s_t[:])
                        # u = r * Sum g   [= lr * Sum g / sqrt(m')]
                        nc.vector.tensor_tensor(
                            out=um2_n[:, half, :, 0:D],
                            in0=ps[:, :, 0:D],
                            in1=r_t[:],
                            op=mybir.AluOpType.mult,
                        )
                    nc.scalar.dma_start(
                        out=um_out[:, j00 * 2 * D:(j00 + J2) * 2 * D],
                        in_=um2_n[:])

    nc.compile()
    return nc


def get_program(rp, **opts):
    key = (rp, tuple(sorted(opts.items())))
    if key not in _program_cache:
        _program_cache[key] = _build_program(rp, **opts)
    return _program_cache[key]


def _choose_rp(max_touched):
    # rp must be a multiple of 12 (JSUB) and 4; 12 covers both.
    rp = -(-max_touched // P)
    rp = -(-rp // 12) * 12
    return rp


def prepare_inputs(gradients, weights, moments, indices, learning_rate,
                   valid_count):
    """Host routing: find touched rows per core, snake-deal them into a
    compact [128, rp] table layout, place first occurrences at identity
    slots and duplicates into per-block overflow chunks."""
    g = np.asarray(gradients, dtype=np.float32)
    w = np.asarray(weights, dtype=np.float32)
    m = np.asarray(moments, dtype=np.float32)
    idx = np.asarray(indices).astype(np.int64)
    vc = int(valid_count)
    lr = float(np.asarray(learning_rate, dtype=np.float32).reshape(-1)[0])

    idxv = idx[:vc]
    gv = g[:vc]
    owner = idxv // VC
    loc = idxv - owner * VC

    per_core = []
    max_touched = 0
    for c in range(NCORES):
        mask = owner == c
        idc = loc[mask]
        gc = gv[mask]
        rows, inv, counts = np.unique(idc, return_inverse=True,
                                      return_counts=True)
        per_core.append((idc, gc, rows, inv, counts))
        max_touched = max(max_touched, len(rows))

    rp = _choose_rp(max_touched)
    # retry with larger rp if overflow slots per block exceed OVF
    for attempt in range(6):
        ok = True
        packed = []
        for c in range(NCORES):
            pc = _pack_core(per_core[c], rp)
            if pc is None:
                ok = False
                break
            packed.append(pc)
        if ok:
            break
        rp += 12
    else:
        return None

    inv_lr2 = 1.0 / (lr * lr)
    eps = 1e-12
    scal = np.array([[inv_lr2, eps]], dtype=np.float32)

    in_maps = []
    unpack_info = []
    for c in range(NCORES):
        h_of, j_of, rows, gb, go, midxo = packed[c]
        mdev = np.zeros((P, rp, D), dtype=np.float16)
        base = c * VC
        mdev[h_of, j_of] = m[base + rows].astype(np.float16)
        in_maps.append({
            "m_in": mdev.reshape(P, rp * D),
            "g_in": gb.reshape(P, rp * D),
            "go_in": go.reshape(P, (rp // 4) * 4 * 2 * D),
            "midxo": midxo,
            "scal": scal,
        })
        unpack_info.append((h_of, j_of, rows))
    return in_maps, rp, unpack_info


def _pack_core(pc, rp):
    """Snake-deal rows into rp blocks; returns (h, j, rows, g_base, g_ovf,
    midx_ovf) or None if an overflow chunk exceeds OVF slots."""
    idc, gc, rows, inv, counts = pc
    T = len(rows)
    if T > P * rp:
        return None
    # deal rows sorted by dup count (desc) so block weights balance
    order = np.argsort(-counts, kind="stable")
    pos = np.arange(T, dtype=np.int64)
    rounds = pos // rp
    k = pos % rp
    j_sorted = np.where(rounds % 2 == 0, k, rp - 1 - k)
    h_sorted = rounds
    # h_of[i], j_of[i] = placement of rows[order[i]] -> map back to row order
    h_of = np.empty(T, dtype=np.int64)
    j_of = np.empty(T, dtype=np.int64)
    h_of[order] = h_sorted
    j_of[order] = j_sorted

    assert rp % 4 == 0
    # occurrences: rank within row (stable sort by row id)
    n = len(idc)
    o = np.argsort(inv, kind="stable")
    starts = np.concatenate(([0], np.cumsum(counts)[:-1]))
    rank = np.empty(n, dtype=np.int64)
    rank[o] = np.arange(n, dtype=np.int64) - starts[inv[o]]

    occ_h = h_of[inv]
    occ_j = j_of[inv]

    g16 = gc.astype(np.float16)
    gb = np.zeros((P, rp, D), dtype=np.float16)
    first = rank == 0
    gb[occ_h[first], occ_j[first]] = g16[first]

    dup = ~first
    dj = occ_j[dup]
    dh = occ_h[dup]
    dg = g16[dup]
    # overflow slots are pooled per 4-block group (128 slots per group)
    dgrp = dj // 4
    db = dj % 4
    do = np.argsort(dgrp, kind="stable")
    gc_ = np.bincount(dgrp, minlength=rp // 4)
    if gc_.max() > P:
        return None
    gstarts = np.concatenate(([0], np.cumsum(gc_)[:-1]))
    slot = np.empty(len(dj), dtype=np.int64)
    slot[do] = np.arange(len(dj), dtype=np.int64) - gstarts[dgrp[do]]

    go = np.zeros((P, rp // 4, 4, 2 * D), dtype=np.float16)
    midxo = np.full((P, rp // 4), -1.0, dtype=np.float16)
    go[slot, dgrp, db, 0:D] = dg
    go[slot, dgrp, db, D:2 * D] = (dg.astype(np.float32) ** 2
                                   ).astype(np.float16)
    midxo[slot, dgrp] = dh.astype(np.float16)
    return h_of, j_of, rows, gb, go, midxo


def assemble_outputs(results, weights, moments, rp, unpack_info):
    w_new = np.array(weights, dtype=np.float32, copy=True)
    m_new = np.array(moments, dtype=np.float32, copy=True)
    for c in range(NCORES):
        h_of, j_of, rows = unpack_info[c]
        um = results[c]["um_out"].reshape(P, rp, 2 * D)
        base = c * VC
        w_new[base + rows] -= um[h_of, j_of, 0:D].astype(np.float32)
        m_new[base + rows] = um[h_of, j_of, D:2 * D].astype(np.float32)
    return w_new, m_new


def _host_reference(gradients, weights, moments, indices, lr, valid_count):
    g = np.asarray(gradients, dtype=np.float64).copy()
    g[int(valid_count):] = 0.0
    idx = np.asarray(indices).astype(np.int64)
    m_new = np.asarray(moments, dtype=np.float64).copy()
    np.add.at(m_new, idx, g * g)
    denom = np.sqrt(m_new[idx]) + 1e-10
    w_new = np.asarray(weights, dtype=np.float64).copy()
    np.add.at(w_new, idx, -lr * g / denom)
    return w_new.astype(np.float32), m_new.astype(np.float32)


def kernel(gradients, weights, moments, indices, learning_rate, valid_count):
    from concourse.bass_utils import run_bass_kernel_spmd

    lr = float(np.asarray(learning_rate, dtype=np.float32).reshape(-1)[0])
    if lr == 0.0:
        # Degenerate: weights unchanged, moments still accumulate g^2.
        g = np.asarray(gradients, dtype=np.float32).copy()
        g[int(valid_count):] = 0.0
        idx = np.asarray(indices).astype(np.int64)
        m_new = np.asarray(moments, dtype=np.float32).copy()
        np.add.at(m_new, idx, g * g)
        return np.asarray(weights, dtype=np.float32).copy(), m_new

    prep = prepare_inputs(
        gradients, weights, moments, indices, learning_rate, valid_count)
    if prep is None:
        # Pathological duplicate distribution the packer can't place
        # (not reachable for uniform indices): host fallback.
        return _host_reference(gradients, weights, moments, indices,
                               lr, valid_count)
    in_maps, rp, unpack_info = prep
    nc = get_program(rp)
    res = run_bass_kernel_spmd(nc, in_maps, core_ids=list(range(NCORES)))
    return assemble_outputs(res.results, weights, moments, rp, unpack_info)


# revision 3
# speedup vs baseline: 1.1869x; 1.1233x over previous
"""Sparse Adagrad (Habana-style) on 8 Trainium2 NeuronCores.

Row-shard the tables across 8 cores by index range (62500 rows each).
Only the TOUCHED rows (~20.6k per core, ~33%) are shipped to the
device, compacted into a [128 partitions x RP blocks] layout chosen by
the host; untouched rows pass through on the host. All device traffic
is fp16 (tolerance is 2e-2; we land ~1e-3).

Compact layout: touched rows are sorted by duplicate-count (desc) and
snake-dealt across RP blocks; the i-th dealt row lands at block
j = snake(i % RP), height h = i // RP, i.e. table position
(partition h, column j). Each row's FIRST gradient occurrence is
stored at base slot h of block j, so the base scatter matrix is the
IDENTITY (constant). Duplicate occurrences (~4.4k/core) are pooled per
4-block PSUM-bank group (up to 128 slots) with a one-hot A_ovf built
on device via is_equal against an iota.

Per PSUM bank (4 blocks, psum[:, 4b:4b+4, 0:128] with [Sum g | Sum g2]
halves per block):
    bank = I @ gsq[4 blocks]            (identity matmul, start=True)
    bank += A_ovf[grp] @ go_diag[grp]   (block-diagonal overflow rhs,
                                         stop=True, same footprint —
                                         HW requires accumulation
                                         groups to open/close with
                                         identical out regions)
then
    m'  = m + Sum g2                          (DVE; GPSIMD can't read PSUM)
    r   = AbsRsqrt(m'*(1/lr^2) + eps)         (ACT)  [= lr*rsqrt(m')]
    u   = r * Sum g                           (DVE)  [= lr*Sum g/sqrt(m')]
outputs [u | m'] per row; the host applies w' = w - u in f32 during
assembly (w never round-trips through fp16). The denominator uses the
fully accumulated m' and is constant across duplicates, so it factors
out of the sum — matching the reference exactly.
"""

import sys

for _p in ("/opt/trn_rl_repo", "/root/.axon_site/_ro/trn_rl_repo"):
    if _p not in sys.path:
        sys.path.insert(0, _p)

import numpy as np

P = 128          # SBUF partitions
D = 64           # embedding dim
NCORES = 8
VC = 62500       # table rows per core
OVF = 32         # overflow slots per block
JSUB = 12        # blocks per sweep iteration (PSUM: [128, 12*128] f32 = 3 banks)

_program_cache = {}


def _build_program(rp, reps=1, rsqrt='act'):
    """rp: number of blocks (table columns) per core; rp % 12 == 0, % 4 == 0."""
    from concourse import bacc, mybir
    import concourse.tile as tile

    nit = rp // JSUB
    assert nit * JSUB == rp and rp % 4 == 0
    rp4 = rp // 4
    f32 = mybir.dt.float32
    f16 = mybir.dt.float16
    nc = bacc.Bacc("TRN2", target_bir_lowering=False, debug=False,
                   num_devices=NCORES)

    # [m | g] interleaved per block (single input stream); host applies
    # w' = w - u during assembly
    mg_in = nc.dram_tensor("mg_in", [P, rp * 2 * D], f16,
                           kind="ExternalInput")
    go_in = nc.dram_tensor("go_in", [P, rp4 * 4 * 2 * D], f16,
                          kind="ExternalInput")
    midxo = nc.dram_tensor("midxo", [P, rp4], f16, kind="ExternalInput")
    scal = nc.dram_tensor("scal", [1, 2], f32, kind="ExternalInput")  # [inv_lr2, eps]
    # output: [u | m'] interleaved per block
    um_out = nc.dram_tensor("um_out", [P, rp * 2 * D], f16,
                            kind="ExternalOutput")

    with tile.TileContext(nc) as tc:
        with tc.tile_pool(name="consts", bufs=1) as consts, \
             tc.tile_pool(name="big", bufs=3) as bigpool, \
             tc.tile_pool(name="sbuf", bufs=4) as pool, \
             tc.tile_pool(name="psum", bufs=2, space="PSUM") as psum:
            # iota along free dim (same in every partition), fp16
            iota_i = consts.tile([P, P], mybir.dt.int32)
            nc.gpsimd.iota(iota_i[:], pattern=[[1, P]], base=0,
                           channel_multiplier=0)
            iota_f = consts.tile([P, P], f16)
            nc.vector.tensor_copy(iota_f[:], iota_i[:])
            # partition index (one value per partition), fp16
            piota_i = consts.tile([P, 1], mybir.dt.int32)
            nc.gpsimd.iota(piota_i[:], pattern=[[1, 1]], base=0,
                           channel_multiplier=1)
            piota_f = consts.tile([P, 1], f16)
            nc.vector.tensor_copy(piota_f[:], piota_i[:])
            # identity matrix [p, f] = (f == p), fp16
            ident = consts.tile([P, P], f16)
            nc.vector.tensor_tensor(
                out=ident[:],
                in0=iota_f[:],
                in1=piota_f[:].to_broadcast((P, P)),
                op=mybir.AluOpType.is_equal,
            )

            inv_lr2 = consts.tile([P, 1], f32)
            nc.sync.dma_start(out=inv_lr2[:],
                              in_=scal[:, 0:1].to_broadcast((P, 1)))
            eps_t = consts.tile([P, 1], f32)
            nc.sync.dma_start(out=eps_t[:],
                              in_=scal[:, 1:2].to_broadcast((P, 1)))

            # overflow gradients, block-diagonal per 4-block group:
            # go_s[slot, grp, db, 0:64]=g, [64:128]=g^2 of that slot if it
            # belongs to sub-block db, else zero. Resident all sweep.
            go_s = consts.tile([P, rp4, 4, 2 * D], f16)
            nc.sync.dma_start(out=go_s[:], in_=go_in[:])
            midxo_s = consts.tile([P, rp4], f16)
            nc.sync.dma_start(out=midxo_s[:], in_=midxo[:])

            # A_ovf[slot, grp, p] = (midxo[slot, grp] == p)
            a_ovf = consts.tile([P, rp4, P], f16)
            nc.vector.tensor_tensor(
                out=a_ovf[:],
                in0=midxo_s[:, :, None].broadcast_to((P, rp4, P)),
                in1=iota_f[:, None, :].broadcast_to((P, rp4, P)),
                op=mybir.AluOpType.is_equal,
            )

            import contextlib

            def _rep_scope():
                return contextlib.nullcontext()

            with _rep_scope():
              for _rep in range(reps):
                NH = 2
                for it2 in range(nit // NH):
                    # DMA at 2-iteration granularity — bigger transfers
                    # amortize DGE latency while staying fine-grained enough
                    # to overlap with compute; compute stays at JSUB blocks
                    # per step (PSUM size).
                    J2 = NH * JSUB
                    j00 = it2 * J2
                    mg2 = bigpool.tile([P, NH, JSUB, 2, D], f16)
                    nc.sync.dma_start(
                        out=mg2[:],
                        in_=mg_in[:, j00 * 2 * D:(j00 + J2) * 2 * D])
                    um2_n = bigpool.tile([P, NH, JSUB, 2 * D], f16)
                    for half in range(NH):
                        j0 = j00 + half * JSUB

                        # [g | g^2] rhs tile: ACT fills both halves
                        gsq = pool.tile([P, JSUB, 2 * D], f16)
                        gb_v = mg2[:, half, :, 1, :]
                        nc.scalar.copy(gsq[:, :, 0:D], gb_v)
                        nc.gpsimd.tensor_tensor(
                            out=gsq[:, :, D:2 * D], in0=gb_v, in1=gb_v,
                            op=mybir.AluOpType.mult)

                        ps = psum.tile([P, JSUB, 2 * D], f32)
                        # Per-region accumulation groups (open and close with
                        # the SAME out footprint — HW/NEFF rejects mismatched
                        # group shapes). start=True lazily marks the whole
                        # 2KB bank pending-zero, so each region's overflow
                        # accumulate must land before the next start touches
                        # that bank: waves of 3 regions in 3 distinct banks
                        # {w, w+4, w+8}, which also share the identity
                        # stationary across 3 matmuls (alternating weights
                        # cost ~3x on PE).
                        # HW requires accumulation groups to open and
                        # close with the SAME out footprint: both the base
                        # (identity) and overflow matmuls cover one whole
                        # PSUM bank (4 blocks, N=512). The overflow rhs is
                        # block-diagonal so one 128-slot matmul serves the
                        # bank's 4 blocks.
                        for b in range(3):
                            nc.tensor.matmul(
                                out=ps[:, 4 * b:4 * (b + 1), :],
                                lhsT=ident[:],
                                rhs=gsq[:, 4 * b:4 * (b + 1), :],
                                start=True, stop=False,
                                skip_group_check=True,
                            )
                        for b in range(3):
                            grp = j0 // 4 + b
                            nc.tensor.matmul(
                                out=ps[:, 4 * b:4 * (b + 1), :],
                                lhsT=a_ovf[:, grp, :],
                                rhs=go_s[:, grp, :, :],
                                start=False, stop=True,
                                skip_group_check=True,
                            )

                        # m' = m + Sum g^2  (psum high half; GPSIMD cannot
                        # read PSUM, so this lives on DVE)
                        nc.vector.tensor_tensor(
                            out=um2_n[:, half, :, D:2 * D],
                            in0=ps[:, :, D:2 * D],
                            in1=mg2[:, half, :, 0, :],
                            op=mybir.AluOpType.add,
                        )
                        # r = 1/sqrt(m'*inv_lr2 + eps) [= lr*rsqrt(m')]
                        # in one ACT op; input >= 0 so the |x| is a no-op.
                        # (CoreSim lacks the fused op: rsqrt='split' swaps in
                        # the equivalent Sqrt + reciprocal chain for sim.)
                        r_t = pool.tile([P, JSUB, D], f32)
                        if rsqrt == 'act':
                            nc.scalar.activation(
                                r_t[:], um2_n[:, half, :, D:2 * D],
                                mybir.ActivationFunctionType.
                                Abs_reciprocal_sqrt,
                                bias=eps_t[:], scale=inv_lr2[:])
                        else:
                            s_t = pool.tile([P, JSUB, D], f32)
                            nc.scalar.activation(
                                s_t[:], um2_n[:, half, :, D:2 * D],
                                mybir.ActivationFunctionType.Sqrt,
                                bias=eps_t[:], scale=inv_lr2[:])
                            nc.vector.reciprocal_approx_fast(out=r_t[:],
                                                             in_=s_t[:])
                        # u = r * Sum g   [= lr * Sum g / sqrt(m')]
                        nc.vector.tensor_tensor(
                            out=um2_n[:, half, :, 0:D],
                            in0=ps[:, :, 0:D],
                            in1=r_t[:],
                            op=mybir.AluOpType.mult,
                        )
                    nc.scalar.dma_start(
                        out=um_out[:, j00 * 2 * D:(j00 + J2) * 2 * D],
                        in_=um2_n[:])

    nc.compile()
    return nc


def get_program(rp, **opts):
    key = (rp, tuple(sorted(opts.items())))
    if key not in _program_cache:
        _program_cache[key] = _build_program(rp, **opts)
    return _program_cache[key]


def _choose_rp(max_touched):
    # rp must be a multiple of 12 (JSUB) and 4; 12 covers both.
    rp = -(-max_touched // P)
    rp = -(-rp // 12) * 12
    return rp


def prepare_inputs(gradients, weights, moments, indices, learning_rate,
                   valid_count):
    """Host routing: find touched rows per core, snake-deal them into a
    compact [128, rp] table layout, place first occurrences at identity
    slots and duplicates into per-block overflow chunks."""
    g = np.asarray(gradients, dtype=np.float32)
    w = np.asarray(weights, dtype=np.float32)
    m = np.asarray(moments, dtype=np.float32)
    idx = np.asarray(indices).astype(np.int64)
    vc = int(valid_count)
    lr = float(np.asarray(learning_rate, dtype=np.float32).reshape(-1)[0])

    idxv = idx[:vc]
    gv = g[:vc]
    owner = idxv // VC
    loc = idxv - owner * VC

    per_core = []
    max_touched = 0
    for c in range(NCORES):
        mask = owner == c
        idc = loc[mask]
        gc = gv[mask]
        rows, inv, counts = np.unique(idc, return_inverse=True,
                                      return_counts=True)
        per_core.append((idc, gc, rows, inv, counts))
        max_touched = max(max_touched, len(rows))

    rp = _choose_rp(max_touched)
    # retry with larger rp if overflow slots per block exceed OVF
    for attempt in range(6):
        ok = True
        packed = []
        for c in range(NCORES):
            pc = _pack_core(per_core[c], rp)
            if pc is None:
                ok = False
                break
            packed.append(pc)
        if ok:
            break
        rp += 12
    else:
        return None

    inv_lr2 = 1.0 / (lr * lr)
    eps = 1e-12
    scal = np.array([[inv_lr2, eps]], dtype=np.float32)

    in_maps = []
    unpack_info = []
    for c in range(NCORES):
        h_of, j_of, rows, gb, go, midxo = packed[c]
        mgdev = np.zeros((P, rp, 2, D), dtype=np.float16)
        base = c * VC
        mgdev[h_of, j_of, 0] = m[base + rows].astype(np.float16)
        mgdev[:, :, 1, :] = gb
        in_maps.append({
            "mg_in": mgdev.reshape(P, rp * 2 * D),
            "go_in": go.reshape(P, (rp // 4) * 4 * 2 * D),
            "midxo": midxo,
            "scal": scal,
        })
        unpack_info.append((h_of, j_of, rows))
    return in_maps, rp, unpack_info


def _pack_core(pc, rp):
    """Snake-deal rows into rp blocks; returns (h, j, rows, g_base, g_ovf,
    midx_ovf) or None if an overflow chunk exceeds OVF slots."""
    idc, gc, rows, inv, counts = pc
    T = len(rows)
    if T > P * rp:
        return None
    # deal rows sorted by dup count (desc) so block weights balance
    order = np.argsort(-counts, kind="stable")
    pos = np.arange(T, dtype=np.int64)
    rounds = pos // rp
    k = pos % rp
    j_sorted = np.where(rounds % 2 == 0, k, rp - 1 - k)
    h_sorted = rounds
    # h_of[i], j_of[i] = placement of rows[order[i]] -> map back to row order
    h_of = np.empty(T, dtype=np.int64)
    j_of = np.empty(T, dtype=np.int64)
    h_of[order] = h_sorted
    j_of[order] = j_sorted

    assert rp % 4 == 0
    # occurrences: rank within row (stable sort by row id)
    n = len(idc)
    o = np.argsort(inv, kind="stable")
    starts = np.concatenate(([0], np.cumsum(counts)[:-1]))
    rank = np.empty(n, dtype=np.int64)
    rank[o] = np.arange(n, dtype=np.int64) - starts[inv[o]]

    occ_h = h_of[inv]
    occ_j = j_of[inv]

    g16 = gc.astype(np.float16)
    gb = np.zeros((P, rp, D), dtype=np.float16)
    first = rank == 0
    gb[occ_h[first], occ_j[first]] = g16[first]

    dup = ~first
    dj = occ_j[dup]
    dh = occ_h[dup]
    dg = g16[dup]
    # overflow slots are pooled per 4-block group (128 slots per group)
    dgrp = dj // 4
    db = dj % 4
    do = np.argsort(dgrp, kind="stable")
    gc_ = np.bincount(dgrp, minlength=rp // 4)
    if gc_.max() > P:
        return None
    gstarts = np.concatenate(([0], np.cumsum(gc_)[:-1]))
    slot = np.empty(len(dj), dtype=np.int64)
    slot[do] = np.arange(len(dj), dtype=np.int64) - gstarts[dgrp[do]]

    go = np.zeros((P, rp // 4, 4, 2 * D), dtype=np.float16)
    midxo = np.full((P, rp // 4), -1.0, dtype=np.float16)
    go[slot, dgrp, db, 0:D] = dg
    go[slot, dgrp, db, D:2 * D] = (dg.astype(np.float32) ** 2
                                   ).astype(np.float16)
    midxo[slot, dgrp] = dh.astype(np.float16)
    return h_of, j_of, rows, gb, go, midxo


def assemble_outputs(results, weights, moments, rp, unpack_info):
    w_new = np.array(weights, dtype=np.float32, copy=True)
    m_new = np.array(moments, dtype=np.float32, copy=True)
    for c in range(NCORES):
        h_of, j_of, rows = unpack_info[c]
        um = results[c]["um_out"].reshape(P, rp, 2 * D)
        base = c * VC
        w_new[base + rows] -= um[h_of, j_of, 0:D].astype(np.float32)
        m_new[base + rows] = um[h_of, j_of, D:2 * D].astype(np.float32)
    return w_new, m_new


def _host_reference(gradients, weights, moments, indices, lr, valid_count):
    g = np.asarray(gradients, dtype=np.float64).copy()
    g[int(valid_count):] = 0.0
    idx = np.asarray(indices).astype(np.int64)
    m_new = np.asarray(moments, dtype=np.float64).copy()
    np.add.at(m_new, idx, g * g)
    denom = np.sqrt(m_new[idx]) + 1e-10
    w_new = np.asarray(weights, dtype=np.float64).copy()
    np.add.at(w_new, idx, -lr * g / denom)
    return w_new.astype(np.float32), m_new.astype(np.float32)


def kernel(gradients, weights, moments, indices, learning_rate, valid_count):
    from concourse.bass_utils import run_bass_kernel_spmd

    lr = float(np.asarray(learning_rate, dtype=np.float32).reshape(-1)[0])
    if lr == 0.0:
        # Degenerate: weights unchanged, moments still accumulate g^2.
        g = np.asarray(gradients, dtype=np.float32).copy()
        g[int(valid_count):] = 0.0
        idx = np.asarray(indices).astype(np.int64)
        m_new = np.asarray(moments, dtype=np.float32).copy()
        np.add.at(m_new, idx, g * g)
        return np.asarray(weights, dtype=np.float32).copy(), m_new

    prep = prepare_inputs(
        gradients, weights, moments, indices, learning_rate, valid_count)
    if prep is None:
        # Pathological duplicate distribution the packer can't place
        # (not reachable for uniform indices): host fallback.
        return _host_reference(gradients, weights, moments, indices,
                               lr, valid_count)
    in_maps, rp, unpack_info = prep
    nc = get_program(rp)
    res = run_bass_kernel_spmd(nc, in_maps, core_ids=list(range(NCORES)))
    return assemble_outputs(res.results, weights, moments, rp, unpack_info)
